# revision 52
# baseline (speedup 1.0000x reference)
"""Two-layer GCN (PyG GCNConv x2, relu between) on 8 trn2 NeuronCores.

Strategy (dst-node partitioned, all on-device math):
  - Nodes are sharded across 8 cores by destination row (12500/core).
  - Layer tables are stored as bf16 PAIR rows: table row k = 256B holding the
    64-feature vectors of nodes 2k and 2k+1.  Tables are computed shard-wise
    on-device, AllGather'ed (bf16, half the fp32 traffic) into a replicated
    DRAM table, and per-edge messages are fetched with GPSIMD dma_gather
    (256B/row, the minimum row size), 4 SWDGE queues round-robin so the Q7
    descriptor generation overlaps across core pairs.
  - Edges are sorted per (dst-block, src-parity) by source pair-row, so each
    gather call covers a narrow table window (dynamic per-call base keeps
    int16 index reach at 32768 pair rows = 65536 nodes), pad slots cluster at
    bin tails as idx=-1 (descgen skips them; a per-core count register keeps
    the ring reservation honest), and each chunk is single-parity so the
    aggregation matmul reads the correct half of the gathered pair row.
  - Segment-sum per 128-dst block is a PE matmul with a one-hot selection
    matrix built on DVE via is_equal against an iota row (pad dstloc=-1 rows
    select nothing); PSUM accumulates across message chunks.
  - Epilogues apply dinv/bias/relu and chain directly into the next layer's
    paired table transform. Final output is produced transposed and unsharded
    on host.

The Bass program is identical on all cores (SPMD); chunk counts per
(block, parity) are the max over cores.
"""

import math
import sys

sys.path.insert(0, "/opt/trn_rl_repo")

import ml_dtypes
import numpy as np


# ---------------------------------------------------------------------------
# configuration
# ---------------------------------------------------------------------------
class Cfg:
    CORES = 8
    N = 100000
    IN_C = 128
    HID = 64
    OUT_C = 40
    NPC = 12500  # nodes per core
    NPC_PAD = 12544  # = 98 * 128
    BLK = 128
    SBB = 4  # dst blocks per superblock (dinv-load granularity)
    SG = 8  # chunks per S-build op
    MAXCH = 5  # max chunks per dma_gather call
    WIN = 32768  # int16 index reach (pair rows) per gather call

    @property
    def NBLK(self):
        return self.NPC_PAD // self.BLK

    @property
    def NSB(self):
        return math.ceil(self.NBLK / self.SBB)

    @property
    def TAB(self):
        return self.NPC_PAD * self.CORES

    @property
    def TAB2(self):
        return self.TAB // 2  # pair rows


# ---------------------------------------------------------------------------
# host-side prep: shard edges, build shared static schedule + per-core arrays
# ---------------------------------------------------------------------------
def _prepare(cfg, edge_index):
    src = np.asarray(edge_index[0], dtype=np.int64)
    dst = np.asarray(edge_index[1], dtype=np.int64)
    loop = np.arange(cfg.N, dtype=np.int64)

    # degree includes the appended self-loops, but the loops themselves are
    # NOT scheduled as gather slots: each core adds its own staged shard rows
    # directly on the PE (constant even/odd spread matmuls per block).
    deg = np.bincount(dst, minlength=cfg.N).astype(np.float32)
    deg += 1.0

    owner = dst // cfg.NPC
    dl_all = dst - owner * cfg.NPC
    srow_all = (src // cfg.NPC) * cfg.NPC_PAD + (src % cfg.NPC)
    par_all = srow_all & 1
    prow_all = srow_all >> 1
    blk_all = dl_all // cfg.BLK

    # per (core, block, parity): edges sorted by source pair-row so each
    # 128-slot chunk covers a narrow table window (per-call dynamic base keeps
    # int16 reach) and per-core pads cluster at bin tails (idx=-1 -> skipped).
    per_cbp = [
        [[None, None] for _ in range(cfg.NBLK)] for _ in range(cfg.CORES)
    ]
    counts = np.zeros((cfg.CORES, cfg.NBLK, 2), dtype=np.int64)
    for c in range(cfg.CORES):
        m = owner == c
        prow, dl, blk, par = prow_all[m], dl_all[m], blk_all[m], par_all[m]
        key = blk * 2 + par
        order = np.lexsort((prow, key))
        prow, dl, key = prow[order], dl[order], key[order]
        bounds = np.searchsorted(key, np.arange(2 * cfg.NBLK + 1))
        for b in range(cfg.NBLK):
            for p in range(2):
                lo, hi = bounds[2 * b + p], bounds[2 * b + p + 1]
                per_cbp[c][b][p] = (prow[lo:hi], dl[lo:hi] % cfg.BLK)
                counts[c, b, p] = hi - lo

    sched = np.ceil(counts.max(axis=0) / cfg.BLK).astype(np.int64)  # [NBLK,2]

    chunk_blocks = []
    chunk_par = []
    bins = []  # (b, p, first_chunk, n_chunks)
    for b in range(cfg.NBLK):
        for p in range(2):
            k = int(sched[b, p])
            bins.append((b, p, len(chunk_blocks), k))
            chunk_blocks.extend([b] * k)
            chunk_par.extend([p] * k)
    nchunk = len(chunk_blocks)
    nslot = nchunk * cfg.BLK

    rows_arr = np.full((cfg.CORES, nslot), -1, dtype=np.int64)
    dloc_arr = np.full((cfg.CORES, nslot), -1, dtype=np.int64)
    for b, p, j0, k in bins:
        pos = j0 * cfg.BLK
        for c in range(cfg.CORES):
            prow, dloc = per_cbp[c][b][p]
            n = len(prow)
            rows_arr[c, pos : pos + n] = prow
            dloc_arr[c, pos : pos + n] = dloc
    del per_cbp

    # greedy call formation within each (block, parity) bin
    calls = []  # (base, first_chunk, n_chunks)
    for b, p, j0, k in bins:
        j = j0
        jend = j0 + k
        while j < jend:
            lo_all, hi_all = None, None
            take = 0
            while take < cfg.MAXCH and j + take < jend:
                s = (j + take) * cfg.BLK
                seg = rows_arr[:, s : s + cfg.BLK]
                seg = seg[seg >= 0]
                if len(seg) == 0:
                    take += 1
                    continue
                lo = min(lo_all, seg.min()) if lo_all is not None else seg.min()
                hi = max(hi_all, seg.max()) if hi_all is not None else seg.max()
                if hi - lo >= cfg.WIN:
                    break
                lo_all, hi_all = lo, hi
                take += 1
            assert take > 0, "single chunk exceeds gather window"
            base = int(lo_all) if lo_all is not None else 0
            calls.append((base, j, take))
            s0, s1 = j * cfg.BLK, (j + take) * cfg.BLK
            seg = rows_arr[:, s0:s1]
            np.subtract(seg, base, out=seg, where=seg >= 0)
            j += take

    assert rows_arr.max() < cfg.WIN
    idx_maps = []
    dstloc_maps = []
    gcnt_maps = []
    for c in range(cfg.CORES):
        idx_arr = rows_arr[c]
        idx_maps.append(np.tile(idx_arr.astype(np.int16).reshape(-1, 16).T, (8, 1)))
        dstloc_maps.append(
            np.ascontiguousarray(
                dloc_arr[c].astype(np.float32).reshape(nchunk, cfg.BLK).T
            )
        )
        # per-call count of real (non-negative) slots: the ucode trims the
        # negative tail, and the decode-side ring reservation must match what
        # the Q7 actually writes -> num_idxs_reg must equal this count.
        cnt = np.empty(len(calls), dtype=np.uint32)
        for k, (_, j0, nch) in enumerate(calls):
            seg = idx_arr[j0 * cfg.BLK : (j0 + nch) * cfg.BLK]
            cnt[k] = int((seg >= 0).sum())
        gcnt_maps.append(cnt.reshape(1, -1))

    # greedy least-loaded SWDGE queue assignment (descgen-bound pairs: span
    # follows the most-loaded pair; plain ci%4 leaves ~8% imbalance). Never
    # assign two consecutive calls to the same pair so dispatch pipelines.
    allc = np.stack([g[0] for g in gcnt_maps])  # [cores, ncalls]
    loads = np.zeros((cfg.CORES, 4))
    qns = []
    last = -1
    for k in range(allc.shape[1]):
        best, bestv = None, None
        for q in range(4):
            if q == last:
                continue
            v = (loads[:, q] + allc[:, k]).max()
            if bestv is None or v < bestv:
                bestv, best = v, q
        qns.append(best)
        loads[:, best] += allc[:, k]
        last = best

    return {
        "qns": qns,
        "deg": deg,
        "sched": sched,
        "chunk_blocks": chunk_blocks,
        "chunk_par": chunk_par,
        "calls": calls,
        "nchunk": nchunk,
        "nslot": nslot,
        "idx_maps": idx_maps,
        "dstloc_maps": dstloc_maps,
        "gcnt_maps": gcnt_maps,
    }


# ---------------------------------------------------------------------------
# device program
# ---------------------------------------------------------------------------
def _build(cfg, chunk_blocks, chunk_par, calls, qns, debug=False, taps=False):
    import concourse.bacc as bacc
    import concourse.mybir as mybir
    import concourse.tile as tile
    from concourse import library_config

    fp32 = mybir.dt.float32
    bf16 = mybir.dt.bfloat16
    AF = mybir.ActivationFunctionType
    ALU = mybir.AluOpType

    nchunk = len(chunk_blocks)
    first_chunk = {}
    last_chunk = {}
    for j, b in enumerate(chunk_blocks):
        first_chunk.setdefault(b, j)
        last_chunk[b] = j
    max_call_ch = max(n for _, _, n in calls)
    HP = cfg.HID // 2  # 32: pair partitions per... (pairs per block = 64)
    PPB = cfg.BLK // 2  # 64 pairs per block

    nc = bacc.Bacc(
        "TRN2",
        target_bir_lowering=False,
        debug=debug,
        num_swdge_queues=4,
        dynamic_dma_scratch_size=32768,
    )

    xT_in = nc.dram_tensor("xTpe", [cfg.IN_C, cfg.NPC_PAD], bf16, kind="ExternalInput")
    W1_in = nc.dram_tensor("W1", [cfg.IN_C, cfg.HID], bf16, kind="ExternalInput")
    W2p_in = nc.dram_tensor("W2p", [cfg.HID, cfg.HID], fp32, kind="ExternalInput")
    b1_in = nc.dram_tensor("b1c", [cfg.HID, 1], fp32, kind="ExternalInput")
    b2_in = nc.dram_tensor("b2c", [cfg.OUT_C, 1], fp32, kind="ExternalInput")
    # dinv broadcast across 128 partitions, node order (for epilogues)
    dinvbc_in = nc.dram_tensor(
        "dinv_bc", [128, cfg.NPC_PAD], fp32, kind="ExternalInput"
    )
    # dinv in pair layout [64 pairs, even|odd], broadcast on 64 partitions,
    # per block 128 wide (for the layer-1 table transform)
    dinvpw_in = nc.dram_tensor(
        "dinv_pw", [PPB, cfg.NPC_PAD], fp32, kind="ExternalInput"
    )
    idx_in = nc.dram_tensor(
        "idxs", [128, (nchunk * cfg.BLK) // 16], mybir.dt.int16, kind="ExternalInput"
    )
    dstloc_in = nc.dram_tensor(
        "dstloc", [cfg.BLK, nchunk], bf16, kind="ExternalInput"
    )
    gcnt_in = nc.dram_tensor(
        "gcnt", [1, len(calls)], mybir.dt.uint32, kind="ExternalInput"
    )
    out_t = nc.dram_tensor(
        "outT", [cfg.OUT_C, cfg.NPC_PAD], fp32, kind="ExternalOutput"
    )
    if taps:
        tap1 = nc.dram_tensor(
            "tap1", [cfg.NPC_PAD // 2, cfg.BLK], bf16, kind="ExternalOutput"
        )
        tap2 = nc.dram_tensor(
            "tap2", [cfg.NPC_PAD // 2, cfg.BLK], bf16, kind="ExternalOutput"
        )

    shard1 = nc.dram_tensor("shard1", [cfg.NPC_PAD // 2, cfg.BLK], bf16)
    shard2 = nc.dram_tensor("shard2", [cfg.NPC_PAD // 2, cfg.BLK], bf16)
    table1 = nc.dram_tensor("table1", [cfg.TAB2, cfg.BLK], bf16, addr_space="Shared")
    table2 = nc.dram_tensor("table2", [cfg.TAB2, cfg.BLK], bf16, addr_space="Shared")
    iota_c = nc.inline_tensor(
        np.tile(np.arange(cfg.BLK, dtype=np.float32), (128, cfg.SG))
        .reshape(128, cfg.SG * cfg.BLK)
        .astype(ml_dtypes.bfloat16),
        name="iota_sg",
    )
    # constant spread matrices for the self-loop contribution: pair row p of
    # the staged shard feeds dst column 2p (even half) / 2p+1 (odd half)
    se_np = np.zeros((PPB, cfg.BLK), np.float32)
    se_np[np.arange(PPB), 2 * np.arange(PPB)] = 1.0
    so_np = np.zeros((PPB, cfg.BLK), np.float32)
    so_np[np.arange(PPB), 2 * np.arange(PPB) + 1] = 1.0
    se_c = nc.inline_tensor(se_np.astype(ml_dtypes.bfloat16), name="spread_even")
    so_c = nc.inline_tensor(so_np.astype(ml_dtypes.bfloat16), name="spread_odd")

    replica = [list(range(cfg.CORES))]

    with tile.TileContext(nc) as tc:
        with (
            tc.tile_pool(name="cst", bufs=1) as cst,
            tc.tile_pool(name="gbp", bufs=10) as gbp,
            tc.tile_pool(name="sp", bufs=4) as sp,
            tc.tile_pool(name="dv", bufs=2) as dv,
            tc.tile_pool(name="ev", bufs=6) as ev,
        ):
            nc.gpsimd.load_library(library_config.mlp)

            # pre-zero the gather pool: tail slots skipped by descgen leave
            # stale tile data that meets a zero S column; NaN/Inf from
            # uninitialized SBUF would still poison 0*NaN, zeros can't.
            for _ in range(10):
                gbt = gbp.tile([128, max_call_ch, cfg.BLK], bf16, tag="gb")
                nc.vector.memset(gbt[:], 0.0)

            # ---- constants ----
            W1t = cst.tile([cfg.IN_C, cfg.HID], bf16)
            nc.sync.dma_start(W1t[:], W1_in[:])
            W2t = cst.tile([cfg.HID, cfg.HID], fp32)
            nc.sync.dma_start(W2t[:], W2p_in[:])
            b1t = cst.tile([cfg.HID, 1], fp32)
            nc.sync.dma_start(b1t[:], b1_in[:])
            b2t = cst.tile([cfg.OUT_C, 1], fp32)
            nc.sync.dma_start(b2t[:], b2_in[:])
            iota = cst.tile([128, cfg.SG * cfg.BLK], bf16)
            nc.sync.dma_start(iota[:], iota_c[:])
            idxt = cst.tile([128, (nchunk * cfg.BLK) // 16], mybir.dt.int16)
            nc.sync.dma_start(idxt[:], idx_in[:])
            dstloct = cst.tile([cfg.BLK, nchunk], bf16)
            nc.sync.dma_start(dstloct[:], dstloc_in[:])
            gcntt = cst.tile([1, len(calls)], mybir.dt.uint32)
            nc.sync.dma_start(gcntt[:], gcnt_in[:])
            set_ = cst.tile([PPB, cfg.BLK], bf16)
            nc.sync.dma_start(set_[:], se_c[:])
            sot = cst.tile([PPB, cfg.BLK], bf16)
            nc.sync.dma_start(sot[:], so_c[:])
            cnt_regs = [nc.gpsimd.alloc_register(f"gather_cnt{i}") for i in range(16)]

            # ---- aggregation layer (shared for both layers) ----
            def agg_layer(layer, table, stag2, stag_self, shard_out=None):
                ch_out = cfg.HID if layer == 1 else cfg.OUT_C
                s_tiles = {}

                def s_for(j):
                    gi = j // cfg.SG
                    if gi not in s_tiles:
                        n = min(cfg.SG, nchunk - gi * cfg.SG)
                        st = sp.tile([128, cfg.SG * cfg.BLK], bf16, tag="s")
                        nc.vector.tensor_tensor(
                            out=st[:].rearrange("p (a b) -> p a b", b=cfg.BLK)[
                                :, :n, :
                            ],
                            in0=iota[:].rearrange("p (a b) -> p a b", b=cfg.BLK)[
                                :, :n, :
                            ],
                            in1=dstloct[:, gi * cfg.SG : gi * cfg.SG + n].to_broadcast(
                                [128, n, cfg.BLK]
                            ),
                            op=ALU.is_equal,
                        )
                        s_tiles[gi] = st
                    return s_tiles[gi], (j % cfg.SG)

                psums = {}
                ci = 0  # call cursor
                for sb in range(cfg.NSB):
                    blo, bhi = sb * cfg.SBB, min((sb + 1) * cfg.SBB, cfg.NBLK)
                    nsb = (bhi - blo) * cfg.BLK
                    dinvrep = dv.tile([128, cfg.SBB * cfg.BLK], fp32, tag="dr")
                    nc.sync.dma_start(
                        dinvrep[:, :nsb],
                        dinvbc_in[:, blo * cfg.BLK : blo * cfg.BLK + nsb],
                    )

                    while ci < len(calls):
                        base, j0, nch = calls[ci]
                        if chunk_blocks[j0] >= bhi:
                            break
                        qn = qns[ci]
                        k = ci
                        ci += 1
                        if k % 16 == 0:
                            n = min(16, len(calls) - k)
                            nc.gpsimd.reg_load(cnt_regs[:n], gcntt[0:1, k : k + n])
                        rows = min(cfg.WIN, cfg.TAB2 - base)
                        gbt = gbp.tile([128, max_call_ch, cfg.BLK], bf16, tag="gb")
                        nc.gpsimd.dma_gather(
                            gbt[:, :nch, :],
                            table[base : base + rows, :],
                            idxt[:, (j0 * cfg.BLK) // 16 : ((j0 + nch) * cfg.BLK) // 16],
                            nch * cfg.BLK,
                            cnt_regs[k % 16],
                            cfg.BLK,
                            queue_num=qn,
                        )
                        for j in range(j0, j0 + nch):
                            b = chunk_blocks[j]
                            if b not in psums:
                                pstile = tc_psum.tile(
                                    [ch_out, cfg.BLK], fp32, tag=f"ps{layer}"
                                )
                                psums[b] = pstile
                                # self-loop contribution from the local shard
                                nc.tensor.matmul(
                                    pstile[:],
                                    lhsT=stag_self[:, b, 0:ch_out],
                                    rhs=set_[:],
                                    start=True,
                                    stop=False,
                                )
                                nc.tensor.matmul(
                                    pstile[:],
                                    lhsT=stag_self[:, b, cfg.HID : cfg.HID + ch_out],
                                    rhs=sot[:],
                                    start=False,
                                    stop=False,
                                )
                            st, kk = s_for(j)
                            off = cfg.HID * chunk_par[j]
                            nc.tensor.matmul(
                                psums[b][:],
                                lhsT=gbt[:, j - j0, off : off + ch_out],
                                rhs=st[:, kk * cfg.BLK : (kk + 1) * cfg.BLK],
                                start=False,
                                stop=(j == last_chunk[b]),
                            )

                    # epilogues for this superblock's blocks
                    for b in range(blo, bhi):
                        off = (b - blo) * cfg.BLK
                        ps = psums.pop(b)
                        if layer == 1:
                            t1 = ev.tile([cfg.HID, cfg.BLK], fp32, tag="t1")
                            nc.vector.tensor_tensor(
                                out=t1[:],
                                in0=ps[:],
                                in1=dinvrep[: cfg.HID, off : off + cfg.BLK],
                                op=ALU.mult,
                            )
                            hr = ev.tile([cfg.HID, cfg.BLK], fp32, tag="hr")
                            nc.scalar.activation(hr[:], t1[:], AF.Relu, bias=b1t[:])
                            # gb columns reordered [even dsts | odd dsts] so the
                            # paired table transform is two contiguous matmuls
                            gb = ev.tile([cfg.HID, cfg.BLK], fp32, tag="gblk")
                            nc.vector.tensor_tensor(
                                out=gb[:, :PPB],
                                in0=hr[:, 0 : cfg.BLK : 2],
                                in1=dinvrep[: cfg.HID, off : off + cfg.BLK : 2],
                                op=ALU.mult,
                            )
                            nc.vector.tensor_tensor(
                                out=gb[:, PPB:],
                                in0=hr[:, 1 : cfg.BLK : 2],
                                in1=dinvrep[: cfg.HID, off + 1 : off + cfg.BLK : 2],
                                op=ALU.mult,
                            )
                            ps2 = tc_ps2.tile([PPB, cfg.BLK], fp32, tag="ps2")
                            nc.tensor.matmul(
                                ps2[:, : cfg.HID],
                                lhsT=gb[:, :PPB],
                                rhs=W2t[:],
                                start=True,
                                stop=True,
                            )
                            nc.tensor.matmul(
                                ps2[:, cfg.HID :],
                                lhsT=gb[:, PPB:],
                                rhs=W2t[:],
                                start=True,
                                stop=True,
                            )
                            nc.scalar.activation(
                                stag2[:, b, :], ps2[:], AF.Copy, bias=0.0
                            )
                        else:
                            t1 = ev.tile([cfg.OUT_C, cfg.BLK], fp32, tag="t2")
                            nc.vector.tensor_tensor(
                                out=t1[:],
                                in0=ps[:],
                                in1=dinvrep[: cfg.OUT_C, off : off + cfg.BLK],
                                op=ALU.mult,
                            )
                            nc.scalar.activation(
                                stag2[:, b * cfg.BLK : (b + 1) * cfg.BLK],
                                t1[:],
                                AF.Identity,
                                bias=b2t[:],
                            )
                    if shard_out is not None:
                        nc.sync.dma_start(
                            shard_out[:, blo:bhi, :], stag2[:, blo:bhi, :]
                        )

            with tc.tile_pool(name="stg2", bufs=1) as st2p:
                stag2 = st2p.tile([PPB, cfg.NBLK, cfg.BLK], bf16)

                with tc.tile_pool(name="stg1", bufs=1) as st1:
                    # ---- layer-1 transform: shard1 pairs = dinv * (x@W1) ----
                    stag1 = st1.tile([PPB, cfg.NBLK, cfg.BLK], bf16)
                    with (
                        tc.tile_pool(name="phA", bufs=3) as pa,
                        tc.tile_pool(name="dpw", bufs=1) as dpw,
                        tc.tile_pool(name="psA", bufs=4, space="PSUM") as psA,
                    ):
                        dpwt = dpw.tile([PPB, cfg.NPC_PAD], fp32)
                        nc.sync.dma_start(dpwt[:], dinvpw_in[:])
                        sh1v = shard1.rearrange("(b p) d -> p b d", p=PPB)
                        for b in range(cfg.NBLK):
                            xc = pa.tile([cfg.IN_C, cfg.BLK], bf16)
                            nc.sync.dma_start(
                                xc[:], xT_in[:, b * cfg.BLK : (b + 1) * cfg.BLK]
                            )
                            ps = psA.tile([PPB, cfg.BLK], fp32)
                            nc.tensor.matmul(
                                ps[:, : cfg.HID],
                                lhsT=xc[:, :PPB],
                                rhs=W1t[:],
                                start=True,
                                stop=True,
                            )
                            nc.tensor.matmul(
                                ps[:, cfg.HID :],
                                lhsT=xc[:, PPB:],
                                rhs=W1t[:],
                                start=True,
                                stop=True,
                            )
                            nc.vector.tensor_tensor(
                                out=stag1[:, b, :],
                                in0=ps[:],
                                in1=dpwt[:, b * cfg.BLK : (b + 1) * cfg.BLK],
                                op=ALU.mult,
                            )
                            # stream the shard out per superblock so the
                            # collective can start right after the last block
                            if b % cfg.SBB == cfg.SBB - 1 or b == cfg.NBLK - 1:
                                blo = (b // cfg.SBB) * cfg.SBB
                                nc.sync.dma_start(
                                    sh1v[:, blo : b + 1, :],
                                    stag1[:, blo : b + 1, :],
                                )

                    nc.gpsimd.collective_compute(
                        "AllGather",
                        mybir.AluOpType.bypass,
                        replica_groups=replica,
                        ins=[shard1[:]],
                        outs=[table1[:]],
                    )
                    if taps:
                        nc.sync.dma_start(tap1[:], shard1[:])

                    # layer 1 aggregation (+ paired table2 transform fused)
                    with (
                        tc.tile_pool(name="ps2p", bufs=2, space="PSUM") as tc_ps2,
                        tc.tile_pool(name="psagg1", bufs=6, space="PSUM") as tc_psum,
                    ):
                        agg_layer(
                            1,
                            table1,
                            stag2,
                            stag1,
                            shard_out=shard2.rearrange("(b p) d -> p b d", p=PPB),
                        )

                nc.gpsimd.collective_compute(
                    "AllGather",
                    mybir.AluOpType.bypass,
                    replica_groups=replica,
                    ins=[shard2[:]],
                    outs=[table2[:]],
                )
                if taps:
                    nc.sync.dma_start(tap2[:], shard2[:])

                # layer 2 aggregation -> transposed output
                with (
                    tc.tile_pool(name="outp", bufs=1) as outp,
                    tc.tile_pool(name="psagg2", bufs=6, space="PSUM") as tc_psum,
                    tc.tile_pool(name="ps2p2", bufs=1, space="PSUM") as tc_ps2,
                ):
                    outT = outp.tile([cfg.OUT_C, cfg.NPC_PAD], fp32)
                    agg_layer(2, table2, outT, stag2)
                    nc.sync.dma_start(out_t[:], outT[:])

    nc.compile()
    return nc


# ---------------------------------------------------------------------------
# public entry point
# ---------------------------------------------------------------------------
def _make_in_maps(cfg, prep, x, W1, b1, W2, b2):
    PPB = cfg.BLK // 2
    W2p = np.zeros((cfg.HID, cfg.HID), np.float32)
    W2p[:, : cfg.OUT_C] = W2
    deg = prep["deg"]
    in_maps = []
    for c in range(cfg.CORES):
        xs = x[c * cfg.NPC : (c + 1) * cfg.NPC]  # [NPC, IN_C]
        xT = np.zeros((cfg.IN_C, cfg.NPC_PAD), np.float32)
        xT[:, : cfg.NPC] = xs.T
        # per-block column permutation [even nodes | odd nodes] so the paired
        # layer-1 transform uses contiguous lhsT slices
        xT_pe = (
            xT.reshape(cfg.IN_C, cfg.NBLK, PPB, 2)
            .transpose(0, 1, 3, 2)
            .reshape(cfg.IN_C, cfg.NPC_PAD)
        )
        # pad nodes: huge degree -> dinv ~ 0 -> pad table rows ~ 0
        dg = np.full(cfg.NPC_PAD, 1e30, np.float32)
        dg[: cfg.NPC] = deg[c * cfg.NPC : (c + 1) * cfg.NPC]
        dinv = (1.0 / np.sqrt(dg)).astype(np.float32)
        dinv_bc = np.ascontiguousarray(
            np.broadcast_to(dinv[None, :], (128, cfg.NPC_PAD))
        )
        # scale tile for the paired layer-1 transform: partition kk = pair-in-
        # block, columns [even-half | odd-half]: value = dinv(block*128 + 2kk
        # + (col >= 64))
        dpair = dinv.reshape(cfg.NBLK, PPB, 2)  # [b, pair, half]
        dinv_pw = np.empty((PPB, cfg.NBLK, 2, PPB), np.float32)
        for h in range(2):
            # [pair kk, block, half h, any col] = dpair[b, kk, h]
            dinv_pw[:, :, h, :] = dpair[:, :, h].T[:, :, None]
        dinv_pw = np.ascontiguousarray(dinv_pw.reshape(PPB, cfg.NPC_PAD))
        in_maps.append(
            {
                "xTpe": np.ascontiguousarray(xT_pe).astype(ml_dtypes.bfloat16),
                "W1": np.asarray(W1, np.float32).astype(ml_dtypes.bfloat16),
                "W2p": W2p,
                "b1c": np.asarray(b1, np.float32).reshape(cfg.HID, 1),
                "b2c": np.asarray(b2, np.float32).reshape(cfg.OUT_C, 1),
                "dinv_bc": dinv_bc,
                "dinv_pw": dinv_pw,
                "idxs": prep["idx_maps"][c],
                "dstloc": prep["dstloc_maps"][c].astype(ml_dtypes.bfloat16),
                "gcnt": prep["gcnt_maps"][c],
            }
        )
    return in_maps


def _run(cfg, inputs, mode="hw", trace=False, taps=False):
    x = np.asarray(inputs["x"], np.float32)
    edge_index = np.asarray(inputs["edge_index"])
    W1 = np.asarray(inputs["W1"], np.float32)
    b1 = np.asarray(inputs["b1"], np.float32)
    W2 = np.asarray(inputs["W2"], np.float32)
    b2 = np.asarray(inputs["b2"], np.float32)

    prep = _prepare(cfg, edge_index)
    nc = _build(
        cfg,
        prep["chunk_blocks"],
        prep["chunk_par"],
        prep["calls"],
        prep["qns"],
        debug=(mode == "sim"),
        taps=taps,
    )
    in_maps = _make_in_maps(cfg, prep, x, W1, b1, W2, b2)

    info = {}
    if mode == "sim":
        from concourse.bass_interp import MultiCoreSim

        sim = MultiCoreSim(nc, cfg.CORES)
        for c in range(cfg.CORES):
            for k, v in in_maps[c].items():
                sim.cores[c].tensor(k)[:] = v
        sim.simulate()
        outs = [sim.cores[c].tensor("outT").copy() for c in range(cfg.CORES)]
        if taps:
            info["taps"] = [
                {k: sim.cores[c].tensor(k).copy() for k in ("tap1", "tap2")}
                for c in range(cfg.CORES)
            ]
    else:
        import concourse.bass_utils as bu

        if trace:
            # avoid the S3 artifact upload in the profile path
            bu.upload_artifacts = lambda d: "(local)"
        r = bu.run_bass_kernel_spmd(
            nc,
            in_maps,
            list(range(cfg.CORES)),
            trace=trace,
            tmpdir=(inputs.get("_tracedir") if trace else None),
        )
        info["exec_time_ns"] = r.exec_time_ns
        info["mean_exec_time_ns"] = r.mean_exec_time_ns
        outs = [r.results[c]["outT"] for c in range(cfg.CORES)]
        if taps:
            info["taps"] = [
                {k: r.results[c][k] for k in ("tap1", "tap2")}
                for c in range(cfg.CORES)
            ]

    out = np.concatenate([o[:, : cfg.NPC].T for o in outs], axis=0)
    return out.astype(np.float32), info


def kernel(**inputs):
    out, _ = _run(Cfg(), inputs, mode="hw")
    return out


# revision 53
# speedup vs baseline: 1.0621x; 1.0621x over previous
"""Two-layer GCN (PyG GCNConv x2, relu between) on 8 trn2 NeuronCores.

Strategy (dst-node partitioned, all on-device math):
  - Nodes are sharded across 8 cores by destination row (12500/core).
  - Layer tables are stored as bf16 PAIR rows: table row k = 256B holding the
    64-feature vectors of nodes 2k and 2k+1.  Tables are computed shard-wise
    on-device, AllGather'ed (bf16, half the fp32 traffic) into a replicated
    DRAM table, and per-edge messages are fetched with GPSIMD dma_gather
    (256B/row, the minimum row size), 4 SWDGE queues round-robin so the Q7
    descriptor generation overlaps across core pairs.
  - Edges are sorted per (dst-block, src-parity) by source pair-row, so each
    gather call covers a narrow table window (dynamic per-call base keeps
    int16 index reach at 32768 pair rows = 65536 nodes), pad slots cluster at
    bin tails as idx=-1 (descgen skips them; a per-core count register keeps
    the ring reservation honest), and each chunk is single-parity so the
    aggregation matmul reads the correct half of the gathered pair row.
  - Segment-sum per 128-dst block is a PE matmul with a one-hot selection
    matrix built on DVE via is_equal against an iota row (pad dstloc=-1 rows
    select nothing); PSUM accumulates across message chunks.
  - Epilogues apply dinv/bias/relu and chain directly into the next layer's
    paired table transform. Final output is produced transposed and unsharded
    on host.

The Bass program is identical on all cores (SPMD); chunk counts per
(block, parity) are the max over cores.
"""

import math
import sys

sys.path.insert(0, "/opt/trn_rl_repo")

import ml_dtypes
import numpy as np


# ---------------------------------------------------------------------------
# configuration
# ---------------------------------------------------------------------------
class Cfg:
    CORES = 8
    N = 100000
    IN_C = 128
    HID = 64
    OUT_C = 40
    NPC = 12500  # nodes per core
    NPC_PAD = 12544  # = 98 * 128
    BLK = 128
    SBB = 4  # dst blocks per superblock (dinv-load granularity)
    SG = 8  # chunks per S-build op
    MAXCH = 5  # max chunks per dma_gather call
    WIN = 32768  # int16 index reach (pair rows) per gather call

    @property
    def NBLK(self):
        return self.NPC_PAD // self.BLK

    @property
    def NSB(self):
        return math.ceil(self.NBLK / self.SBB)

    @property
    def TAB(self):
        return self.NPC_PAD * self.CORES

    @property
    def TAB2(self):
        return self.TAB // 2  # pair rows


# ---------------------------------------------------------------------------
# host-side prep: shard edges, build shared static schedule + per-core arrays
# ---------------------------------------------------------------------------
def _prepare(cfg, edge_index):
    src = np.asarray(edge_index[0], dtype=np.int64)
    dst = np.asarray(edge_index[1], dtype=np.int64)
    loop = np.arange(cfg.N, dtype=np.int64)

    # degree includes the appended self-loops, but the loops themselves are
    # NOT scheduled as gather slots: each core adds its own staged shard rows
    # directly on the PE (constant even/odd spread matmuls per block).
    deg = np.bincount(dst, minlength=cfg.N).astype(np.float32)
    deg += 1.0

    owner = dst // cfg.NPC
    dl_all = dst - owner * cfg.NPC
    srow_all = (src // cfg.NPC) * cfg.NPC_PAD + (src % cfg.NPC)
    par_all = srow_all & 1
    prow_all = srow_all >> 1
    blk_all = dl_all // cfg.BLK

    # per (core, block, parity): edges sorted by source pair-row so each
    # 128-slot chunk covers a narrow table window (per-call dynamic base keeps
    # int16 reach) and per-core pads cluster at bin tails (idx=-1 -> skipped).
    per_cbp = [
        [[None, None] for _ in range(cfg.NBLK)] for _ in range(cfg.CORES)
    ]
    counts = np.zeros((cfg.CORES, cfg.NBLK, 2), dtype=np.int64)
    for c in range(cfg.CORES):
        m = owner == c
        prow, dl, blk, par = prow_all[m], dl_all[m], blk_all[m], par_all[m]
        key = blk * 2 + par
        order = np.lexsort((prow, key))
        prow, dl, key = prow[order], dl[order], key[order]
        bounds = np.searchsorted(key, np.arange(2 * cfg.NBLK + 1))
        for b in range(cfg.NBLK):
            for p in range(2):
                lo, hi = bounds[2 * b + p], bounds[2 * b + p + 1]
                per_cbp[c][b][p] = (prow[lo:hi], dl[lo:hi] % cfg.BLK)
                counts[c, b, p] = hi - lo

    sched = np.ceil(counts.max(axis=0) / cfg.BLK).astype(np.int64)  # [NBLK,2]

    chunk_blocks = []
    chunk_par = []
    bins = []  # (b, p, first_chunk, n_chunks)
    for b in range(cfg.NBLK):
        for p in range(2):
            k = int(sched[b, p])
            bins.append((b, p, len(chunk_blocks), k))
            chunk_blocks.extend([b] * k)
            chunk_par.extend([p] * k)
    nchunk = len(chunk_blocks)
    nslot = nchunk * cfg.BLK

    rows_arr = np.full((cfg.CORES, nslot), -1, dtype=np.int64)
    dloc_arr = np.full((cfg.CORES, nslot), -1, dtype=np.int64)
    for b, p, j0, k in bins:
        pos = j0 * cfg.BLK
        for c in range(cfg.CORES):
            prow, dloc = per_cbp[c][b][p]
            n = len(prow)
            rows_arr[c, pos : pos + n] = prow
            dloc_arr[c, pos : pos + n] = dloc
    del per_cbp

    # greedy call formation within each (block, parity) bin
    calls = []  # (base, first_chunk, n_chunks)
    for b, p, j0, k in bins:
        j = j0
        jend = j0 + k
        while j < jend:
            lo_all, hi_all = None, None
            take = 0
            while take < cfg.MAXCH and j + take < jend:
                s = (j + take) * cfg.BLK
                seg = rows_arr[:, s : s + cfg.BLK]
                seg = seg[seg >= 0]
                if len(seg) == 0:
                    take += 1
                    continue
                lo = min(lo_all, seg.min()) if lo_all is not None else seg.min()
                hi = max(hi_all, seg.max()) if hi_all is not None else seg.max()
                if hi - lo >= cfg.WIN:
                    break
                lo_all, hi_all = lo, hi
                take += 1
            assert take > 0, "single chunk exceeds gather window"
            base = int(lo_all) if lo_all is not None else 0
            calls.append((base, j, take))
            s0, s1 = j * cfg.BLK, (j + take) * cfg.BLK
            seg = rows_arr[:, s0:s1]
            np.subtract(seg, base, out=seg, where=seg >= 0)
            j += take

    assert rows_arr.max() < cfg.WIN
    idx_maps = []
    dstloc_maps = []
    gcnt_maps = []
    for c in range(cfg.CORES):
        idx_arr = rows_arr[c]
        idx_maps.append(np.tile(idx_arr.astype(np.int16).reshape(-1, 16).T, (8, 1)))
        dstloc_maps.append(
            np.ascontiguousarray(
                dloc_arr[c].astype(np.float32).reshape(nchunk, cfg.BLK).T
            )
        )
        # per-call count of real (non-negative) slots: the ucode trims the
        # negative tail, and the decode-side ring reservation must match what
        # the Q7 actually writes -> num_idxs_reg must equal this count.
        cnt = np.empty(len(calls), dtype=np.uint32)
        for k, (_, j0, nch) in enumerate(calls):
            seg = idx_arr[j0 * cfg.BLK : (j0 + nch) * cfg.BLK]
            cnt[k] = int((seg >= 0).sum())
        gcnt_maps.append(cnt.reshape(1, -1))

    # greedy least-loaded SWDGE queue assignment (descgen-bound pairs: span
    # follows the most-loaded pair; plain ci%4 leaves ~8% imbalance). Never
    # assign two consecutive calls to the same pair so dispatch pipelines.
    mean_cnt = np.mean([g[0] for g in gcnt_maps], axis=0)
    loads = np.zeros(4)
    qns = []
    last = -1
    for c in mean_cnt:
        order = np.argsort(loads)
        pick = int(order[0]) if int(order[0]) != last else int(order[1])
        qns.append(pick)
        loads[pick] += c
        last = pick

    return {
        "qns": qns,
        "deg": deg,
        "sched": sched,
        "chunk_blocks": chunk_blocks,
        "chunk_par": chunk_par,
        "calls": calls,
        "nchunk": nchunk,
        "nslot": nslot,
        "idx_maps": idx_maps,
        "dstloc_maps": dstloc_maps,
        "gcnt_maps": gcnt_maps,
    }


# ---------------------------------------------------------------------------
# device program
# ---------------------------------------------------------------------------
def _build(cfg, chunk_blocks, chunk_par, calls, qns, debug=False, taps=False):
    import concourse.bacc as bacc
    import concourse.mybir as mybir
    import concourse.tile as tile
    from concourse import library_config

    fp32 = mybir.dt.float32
    bf16 = mybir.dt.bfloat16
    AF = mybir.ActivationFunctionType
    ALU = mybir.AluOpType

    nchunk = len(chunk_blocks)
    first_chunk = {}
    last_chunk = {}
    for j, b in enumerate(chunk_blocks):
        first_chunk.setdefault(b, j)
        last_chunk[b] = j
    max_call_ch = max(n for _, _, n in calls)
    HP = cfg.HID // 2  # 32: pair partitions per... (pairs per block = 64)
    PPB = cfg.BLK // 2  # 64 pairs per block

    nc = bacc.Bacc(
        "TRN2",
        target_bir_lowering=False,
        debug=debug,
        num_swdge_queues=4,
        dynamic_dma_scratch_size=32768,
    )

    xT_in = nc.dram_tensor("xTpe", [cfg.IN_C, cfg.NPC_PAD], bf16, kind="ExternalInput")
    W1_in = nc.dram_tensor("W1", [cfg.IN_C, cfg.HID], bf16, kind="ExternalInput")
    W2p_in = nc.dram_tensor("W2p", [cfg.HID, cfg.HID], fp32, kind="ExternalInput")
    b1_in = nc.dram_tensor("b1c", [cfg.HID, 1], fp32, kind="ExternalInput")
    b2_in = nc.dram_tensor("b2c", [cfg.OUT_C, 1], fp32, kind="ExternalInput")
    # dinv broadcast across 128 partitions, node order (for epilogues)
    dinvbc_in = nc.dram_tensor(
        "dinv_bc", [128, cfg.NPC_PAD], fp32, kind="ExternalInput"
    )
    # dinv in pair layout [64 pairs, even|odd], broadcast on 64 partitions,
    # per block 128 wide (for the layer-1 table transform)
    dinvpw_in = nc.dram_tensor(
        "dinv_pw", [PPB, cfg.NPC_PAD], fp32, kind="ExternalInput"
    )
    idx_in = nc.dram_tensor(
        "idxs", [128, (nchunk * cfg.BLK) // 16], mybir.dt.int16, kind="ExternalInput"
    )
    dstloc_in = nc.dram_tensor(
        "dstloc", [cfg.BLK, nchunk], bf16, kind="ExternalInput"
    )
    gcnt_in = nc.dram_tensor(
        "gcnt", [1, len(calls)], mybir.dt.uint32, kind="ExternalInput"
    )
    out_t = nc.dram_tensor(
        "outT", [cfg.OUT_C, cfg.NPC_PAD], fp32, kind="ExternalOutput"
    )
    if taps:
        tap1 = nc.dram_tensor(
            "tap1", [cfg.NPC_PAD // 2, cfg.BLK], bf16, kind="ExternalOutput"
        )
        tap2 = nc.dram_tensor(
            "tap2", [cfg.NPC_PAD // 2, cfg.BLK], bf16, kind="ExternalOutput"
        )

    shard1 = nc.dram_tensor("shard1", [cfg.NPC_PAD // 2, cfg.BLK], bf16)
    shard2 = nc.dram_tensor("shard2", [cfg.NPC_PAD // 2, cfg.BLK], bf16)
    table1 = nc.dram_tensor("table1", [cfg.TAB2, cfg.BLK], bf16, addr_space="Shared")
    table2 = nc.dram_tensor("table2", [cfg.TAB2, cfg.BLK], bf16, addr_space="Shared")
    iota_c = nc.inline_tensor(
        np.tile(np.arange(cfg.BLK, dtype=np.float32), (128, cfg.SG))
        .reshape(128, cfg.SG * cfg.BLK)
        .astype(ml_dtypes.bfloat16),
        name="iota_sg",
    )
    # constant spread matrices for the self-loop contribution: pair row p of
    # the staged shard feeds dst column 2p (even half) / 2p+1 (odd half)
    se_np = np.zeros((PPB, cfg.BLK), np.float32)
    se_np[np.arange(PPB), 2 * np.arange(PPB)] = 1.0
    so_np = np.zeros((PPB, cfg.BLK), np.float32)
    so_np[np.arange(PPB), 2 * np.arange(PPB) + 1] = 1.0
    se_c = nc.inline_tensor(se_np.astype(ml_dtypes.bfloat16), name="spread_even")
    so_c = nc.inline_tensor(so_np.astype(ml_dtypes.bfloat16), name="spread_odd")

    replica = [list(range(cfg.CORES))]

    with tile.TileContext(nc) as tc:
        with (
            tc.tile_pool(name="cst", bufs=1) as cst,
            tc.tile_pool(name="gbp", bufs=10) as gbp,
            tc.tile_pool(name="sp", bufs=4) as sp,
            tc.tile_pool(name="dv", bufs=2) as dv,
            tc.tile_pool(name="ev", bufs=6) as ev,
        ):
            nc.gpsimd.load_library(library_config.mlp)

            # pre-zero the gather pool: tail slots skipped by descgen leave
            # stale tile data that meets a zero S column; NaN/Inf from
            # uninitialized SBUF would still poison 0*NaN, zeros can't.
            for _ in range(10):
                gbt = gbp.tile([128, max_call_ch, cfg.BLK], bf16, tag="gb")
                nc.vector.memset(gbt[:], 0.0)

            # ---- constants ----
            W1t = cst.tile([cfg.IN_C, cfg.HID], bf16)
            nc.sync.dma_start(W1t[:], W1_in[:])
            W2t = cst.tile([cfg.HID, cfg.HID], fp32)
            nc.sync.dma_start(W2t[:], W2p_in[:])
            b1t = cst.tile([cfg.HID, 1], fp32)
            nc.sync.dma_start(b1t[:], b1_in[:])
            b2t = cst.tile([cfg.OUT_C, 1], fp32)
            nc.sync.dma_start(b2t[:], b2_in[:])
            iota = cst.tile([128, cfg.SG * cfg.BLK], bf16)
            nc.sync.dma_start(iota[:], iota_c[:])
            idxt = cst.tile([128, (nchunk * cfg.BLK) // 16], mybir.dt.int16)
            nc.sync.dma_start(idxt[:], idx_in[:])
            dstloct = cst.tile([cfg.BLK, nchunk], bf16)
            nc.sync.dma_start(dstloct[:], dstloc_in[:])
            gcntt = cst.tile([1, len(calls)], mybir.dt.uint32)
            nc.sync.dma_start(gcntt[:], gcnt_in[:])
            set_ = cst.tile([PPB, cfg.BLK], bf16)
            nc.sync.dma_start(set_[:], se_c[:])
            sot = cst.tile([PPB, cfg.BLK], bf16)
            nc.sync.dma_start(sot[:], so_c[:])
            cnt_regs = [nc.gpsimd.alloc_register(f"gather_cnt{i}") for i in range(16)]

            # ---- aggregation layer (shared for both layers) ----
            def agg_layer(layer, table, stag2, stag_self, shard_out=None):
                ch_out = cfg.HID if layer == 1 else cfg.OUT_C
                s_tiles = {}

                def s_for(j):
                    gi = j // cfg.SG
                    if gi not in s_tiles:
                        n = min(cfg.SG, nchunk - gi * cfg.SG)
                        st = sp.tile([128, cfg.SG * cfg.BLK], bf16, tag="s")
                        nc.vector.tensor_tensor(
                            out=st[:].rearrange("p (a b) -> p a b", b=cfg.BLK)[
                                :, :n, :
                            ],
                            in0=iota[:].rearrange("p (a b) -> p a b", b=cfg.BLK)[
                                :, :n, :
                            ],
                            in1=dstloct[:, gi * cfg.SG : gi * cfg.SG + n].to_broadcast(
                                [128, n, cfg.BLK]
                            ),
                            op=ALU.is_equal,
                        )
                        s_tiles[gi] = st
                    return s_tiles[gi], (j % cfg.SG)

                psums = {}
                ci = 0  # call cursor
                for sb in range(cfg.NSB):
                    blo, bhi = sb * cfg.SBB, min((sb + 1) * cfg.SBB, cfg.NBLK)
                    nsb = (bhi - blo) * cfg.BLK
                    dinvrep = dv.tile([128, cfg.SBB * cfg.BLK], fp32, tag="dr")
                    nc.sync.dma_start(
                        dinvrep[:, :nsb],
                        dinvbc_in[:, blo * cfg.BLK : blo * cfg.BLK + nsb],
                    )

                    while ci < len(calls):
                        base, j0, nch = calls[ci]
                        if chunk_blocks[j0] >= bhi:
                            break
                        qn = qns[ci]
                        k = ci
                        ci += 1
                        if k % 16 == 0:
                            n = min(16, len(calls) - k)
                            nc.gpsimd.reg_load(cnt_regs[:n], gcntt[0:1, k : k + n])
                        rows = min(cfg.WIN, cfg.TAB2 - base)
                        gbt = gbp.tile([128, max_call_ch, cfg.BLK], bf16, tag="gb")
                        nc.gpsimd.dma_gather(
                            gbt[:, :nch, :],
                            table[base : base + rows, :],
                            idxt[:, (j0 * cfg.BLK) // 16 : ((j0 + nch) * cfg.BLK) // 16],
                            nch * cfg.BLK,
                            cnt_regs[k % 16],
                            cfg.BLK,
                            queue_num=qn,
                        )
                        for j in range(j0, j0 + nch):
                            b = chunk_blocks[j]
                            if b not in psums:
                                pstile = tc_psum.tile(
                                    [ch_out, cfg.BLK], fp32, tag=f"ps{layer}"
                                )
                                psums[b] = pstile
                                # self-loop contribution from the local shard
                                nc.tensor.matmul(
                                    pstile[:],
                                    lhsT=stag_self[:, b, 0:ch_out],
                                    rhs=set_[:],
                                    start=True,
                                    stop=False,
                                )
                                nc.tensor.matmul(
                                    pstile[:],
                                    lhsT=stag_self[:, b, cfg.HID : cfg.HID + ch_out],
                                    rhs=sot[:],
                                    start=False,
                                    stop=False,
                                )
                            st, kk = s_for(j)
                            off = cfg.HID * chunk_par[j]
                            nc.tensor.matmul(
                                psums[b][:],
                                lhsT=gbt[:, j - j0, off : off + ch_out],
                                rhs=st[:, kk * cfg.BLK : (kk + 1) * cfg.BLK],
                                start=False,
                                stop=(j == last_chunk[b]),
                            )

                    # epilogues for this superblock's blocks
                    for b in range(blo, bhi):
                        off = (b - blo) * cfg.BLK
                        ps = psums.pop(b)
                        if layer == 1:
                            t1 = ev.tile([cfg.HID, cfg.BLK], fp32, tag="t1")
                            nc.vector.tensor_tensor(
                                out=t1[:],
                                in0=ps[:],
                                in1=dinvrep[: cfg.HID, off : off + cfg.BLK],
                                op=ALU.mult,
                            )
                            hr = ev.tile([cfg.HID, cfg.BLK], fp32, tag="hr")
                            nc.scalar.activation(hr[:], t1[:], AF.Relu, bias=b1t[:])
                            # gb columns reordered [even dsts | odd dsts] so the
                            # paired table transform is two contiguous matmuls
                            gb = ev.tile([cfg.HID, cfg.BLK], fp32, tag="gblk")
                            nc.vector.tensor_tensor(
                                out=gb[:, :PPB],
                                in0=hr[:, 0 : cfg.BLK : 2],
                                in1=dinvrep[: cfg.HID, off : off + cfg.BLK : 2],
                                op=ALU.mult,
                            )
                            nc.vector.tensor_tensor(
                                out=gb[:, PPB:],
                                in0=hr[:, 1 : cfg.BLK : 2],
                                in1=dinvrep[: cfg.HID, off + 1 : off + cfg.BLK : 2],
                                op=ALU.mult,
                            )
                            ps2 = tc_ps2.tile([PPB, cfg.BLK], fp32, tag="ps2")
                            nc.tensor.matmul(
                                ps2[:, : cfg.HID],
                                lhsT=gb[:, :PPB],
                                rhs=W2t[:],
                                start=True,
                                stop=True,
                            )
                            nc.tensor.matmul(
                                ps2[:, cfg.HID :],
                                lhsT=gb[:, PPB:],
                                rhs=W2t[:],
                                start=True,
                                stop=True,
                            )
                            nc.scalar.activation(
                                stag2[:, b, :], ps2[:], AF.Copy, bias=0.0
                            )
                        else:
                            t1 = ev.tile([cfg.OUT_C, cfg.BLK], fp32, tag="t2")
                            nc.vector.tensor_tensor(
                                out=t1[:],
                                in0=ps[:],
                                in1=dinvrep[: cfg.OUT_C, off : off + cfg.BLK],
                                op=ALU.mult,
                            )
                            nc.scalar.activation(
                                stag2[:, b * cfg.BLK : (b + 1) * cfg.BLK],
                                t1[:],
                                AF.Identity,
                                bias=b2t[:],
                            )
                    if shard_out is not None:
                        nc.sync.dma_start(
                            shard_out[:, blo:bhi, :], stag2[:, blo:bhi, :]
                        )

            with tc.tile_pool(name="stg2", bufs=1) as st2p:
                stag2 = st2p.tile([PPB, cfg.NBLK, cfg.BLK], bf16)

                with tc.tile_pool(name="stg1", bufs=1) as st1:
                    # ---- layer-1 transform: shard1 pairs = dinv * (x@W1) ----
                    stag1 = st1.tile([PPB, cfg.NBLK, cfg.BLK], bf16)
                    with (
                        tc.tile_pool(name="phA", bufs=3) as pa,
                        tc.tile_pool(name="dpw", bufs=1) as dpw,
                        tc.tile_pool(name="psA", bufs=4, space="PSUM") as psA,
                    ):
                        dpwt = dpw.tile([PPB, cfg.NPC_PAD], fp32)
                        nc.sync.dma_start(dpwt[:], dinvpw_in[:])
                        sh1v = shard1.rearrange("(b p) d -> p b d", p=PPB)
                        for b in range(cfg.NBLK):
                            if b % 8 == 0:
                                nbb = min(8, cfg.NBLK - b)
                                xc8 = pa.tile([cfg.IN_C, 8 * cfg.BLK], bf16)
                                nc.sync.dma_start(
                                    xc8[:, : nbb * cfg.BLK],
                                    xT_in[
                                        :,
                                        b * cfg.BLK : (b + nbb) * cfg.BLK,
                                    ],
                                )
                            xo = (b % 8) * cfg.BLK
                            ps = psA.tile([PPB, cfg.BLK], fp32)
                            nc.tensor.matmul(
                                ps[:, : cfg.HID],
                                lhsT=xc8[:, xo : xo + PPB],
                                rhs=W1t[:],
                                start=True,
                                stop=True,
                            )
                            nc.tensor.matmul(
                                ps[:, cfg.HID :],
                                lhsT=xc8[:, xo + PPB : xo + cfg.BLK],
                                rhs=W1t[:],
                                start=True,
                                stop=True,
                            )
                            nc.vector.tensor_tensor(
                                out=stag1[:, b, :],
                                in0=ps[:],
                                in1=dpwt[:, b * cfg.BLK : (b + 1) * cfg.BLK],
                                op=ALU.mult,
                            )
                            # stream the shard out per superblock so the
                            # collective can start right after the last block
                            if b % cfg.SBB == cfg.SBB - 1 or b == cfg.NBLK - 1:
                                blo = (b // cfg.SBB) * cfg.SBB
                                nc.sync.dma_start(
                                    sh1v[:, blo : b + 1, :],
                                    stag1[:, blo : b + 1, :],
                                )

                    nc.gpsimd.collective_compute(
                        "AllGather",
                        mybir.AluOpType.bypass,
                        replica_groups=replica,
                        ins=[shard1[:]],
                        outs=[table1[:]],
                    )
                    if taps:
                        nc.sync.dma_start(tap1[:], shard1[:])

                    # layer 1 aggregation (+ paired table2 transform fused)
                    with (
                        tc.tile_pool(name="ps2p", bufs=2, space="PSUM") as tc_ps2,
                        tc.tile_pool(name="psagg1", bufs=6, space="PSUM") as tc_psum,
                    ):
                        agg_layer(
                            1,
                            table1,
                            stag2,
                            stag1,
                            shard_out=shard2.rearrange("(b p) d -> p b d", p=PPB),
                        )

                nc.gpsimd.collective_compute(
                    "AllGather",
                    mybir.AluOpType.bypass,
                    replica_groups=replica,
                    ins=[shard2[:]],
                    outs=[table2[:]],
                )
                if taps:
                    nc.sync.dma_start(tap2[:], shard2[:])

                # layer 2 aggregation -> transposed output
                with (
                    tc.tile_pool(name="outp", bufs=1) as outp,
                    tc.tile_pool(name="psagg2", bufs=6, space="PSUM") as tc_psum,
                    tc.tile_pool(name="ps2p2", bufs=1, space="PSUM") as tc_ps2,
                ):
                    outT = outp.tile([cfg.OUT_C, cfg.NPC_PAD], fp32)
                    agg_layer(2, table2, outT, stag2)
                    nc.sync.dma_start(out_t[:], outT[:])

    nc.compile()
    return nc


# ---------------------------------------------------------------------------
# public entry point
# ---------------------------------------------------------------------------
def _make_in_maps(cfg, prep, x, W1, b1, W2, b2):
    PPB = cfg.BLK // 2
    W2p = np.zeros((cfg.HID, cfg.HID), np.float32)
    W2p[:, : cfg.OUT_C] = W2
    deg = prep["deg"]
    in_maps = []
    for c in range(cfg.CORES):
        xs = x[c * cfg.NPC : (c + 1) * cfg.NPC]  # [NPC, IN_C]
        xT = np.zeros((cfg.IN_C, cfg.NPC_PAD), np.float32)
        xT[:, : cfg.NPC] = xs.T
        # per-block column permutation [even nodes | odd nodes] so the paired
        # layer-1 transform uses contiguous lhsT slices
        xT_pe = (
            xT.reshape(cfg.IN_C, cfg.NBLK, PPB, 2)
            .transpose(0, 1, 3, 2)
            .reshape(cfg.IN_C, cfg.NPC_PAD)
        )
        # pad nodes: huge degree -> dinv ~ 0 -> pad table rows ~ 0
        dg = np.full(cfg.NPC_PAD, 1e30, np.float32)
        dg[: cfg.NPC] = deg[c * cfg.NPC : (c + 1) * cfg.NPC]
        dinv = (1.0 / np.sqrt(dg)).astype(np.float32)
        dinv_bc = np.ascontiguousarray(
            np.broadcast_to(dinv[None, :], (128, cfg.NPC_PAD))
        )
        # scale tile for the paired layer-1 transform: partition kk = pair-in-
        # block, columns [even-half | odd-half]: value = dinv(block*128 + 2kk
        # + (col >= 64))
        dpair = dinv.reshape(cfg.NBLK, PPB, 2)  # [b, pair, half]
        dinv_pw = np.empty((PPB, cfg.NBLK, 2, PPB), np.float32)
        for h in range(2):
            # [pair kk, block, half h, any col] = dpair[b, kk, h]
            dinv_pw[:, :, h, :] = dpair[:, :, h].T[:, :, None]
        dinv_pw = np.ascontiguousarray(dinv_pw.reshape(PPB, cfg.NPC_PAD))
        in_maps.append(
            {
                "xTpe": np.ascontiguousarray(xT_pe).astype(ml_dtypes.bfloat16),
                "W1": np.asarray(W1, np.float32).astype(ml_dtypes.bfloat16),
                "W2p": W2p,
                "b1c": np.asarray(b1, np.float32).reshape(cfg.HID, 1),
                "b2c": np.asarray(b2, np.float32).reshape(cfg.OUT_C, 1),
                "dinv_bc": dinv_bc,
                "dinv_pw": dinv_pw,
                "idxs": prep["idx_maps"][c],
                "dstloc": prep["dstloc_maps"][c].astype(ml_dtypes.bfloat16),
                "gcnt": prep["gcnt_maps"][c],
            }
        )
    return in_maps


def _run(cfg, inputs, mode="hw", trace=False, taps=False):
    x = np.asarray(inputs["x"], np.float32)
    edge_index = np.asarray(inputs["edge_index"])
    W1 = np.asarray(inputs["W1"], np.float32)
    b1 = np.asarray(inputs["b1"], np.float32)
    W2 = np.asarray(inputs["W2"], np.float32)
    b2 = np.asarray(inputs["b2"], np.float32)

    prep = _prepare(cfg, edge_index)
    nc = _build(
        cfg,
        prep["chunk_blocks"],
        prep["chunk_par"],
        prep["calls"],
        prep["qns"],
        debug=(mode == "sim"),
        taps=taps,
    )
    in_maps = _make_in_maps(cfg, prep, x, W1, b1, W2, b2)

    info = {}
    if mode == "sim":
        from concourse.bass_interp import MultiCoreSim

        sim = MultiCoreSim(nc, cfg.CORES)
        for c in range(cfg.CORES):
            for k, v in in_maps[c].items():
                sim.cores[c].tensor(k)[:] = v
        sim.simulate()
        outs = [sim.cores[c].tensor("outT").copy() for c in range(cfg.CORES)]
        if taps:
            info["taps"] = [
                {k: sim.cores[c].tensor(k).copy() for k in ("tap1", "tap2")}
                for c in range(cfg.CORES)
            ]
    else:
        import concourse.bass_utils as bu

        if trace:
            # avoid the S3 artifact upload in the profile path
            bu.upload_artifacts = lambda d: "(local)"
        r = bu.run_bass_kernel_spmd(
            nc,
            in_maps,
            list(range(cfg.CORES)),
            trace=trace,
            tmpdir=(inputs.get("_tracedir") if trace else None),
        )
        info["exec_time_ns"] = r.exec_time_ns
        info["mean_exec_time_ns"] = r.mean_exec_time_ns
        outs = [r.results[c]["outT"] for c in range(cfg.CORES)]
        if taps:
            info["taps"] = [
                {k: r.results[c][k] for k in ("tap1", "tap2")}
                for c in range(cfg.CORES)
            ]

    out = np.concatenate([o[:, : cfg.NPC].T for o in outs], axis=0)
    return out.astype(np.float32), info


def kernel(**inputs):
    out, _ = _run(Cfg(), inputs, mode="hw")
    return out


# revision 54
# speedup vs baseline: 1.0631x; 1.0009x over previous
"""Two-layer GCN (PyG GCNConv x2, relu between) on 8 trn2 NeuronCores.

Strategy (dst-node partitioned, all on-device math):
  - Nodes are sharded across 8 cores by destination row (12500/core).
  - Layer tables are stored as bf16 PAIR rows: table row k = 256B holding the
    64-feature vectors of nodes 2k and 2k+1.  Tables are computed shard-wise
    on-device, AllGather'ed (bf16, half the fp32 traffic) into a replicated
    DRAM table, and per-edge messages are fetched with GPSIMD dma_gather
    (256B/row, the minimum row size), 4 SWDGE queues round-robin so the Q7
    descriptor generation overlaps across core pairs.
  - Edges are sorted per (dst-block, src-parity) by source pair-row, so each
    gather call covers a narrow table window (dynamic per-call base keeps
    int16 index reach at 32768 pair rows = 65536 nodes), pad slots cluster at
    bin tails as idx=-1 (descgen skips them; a per-core count register keeps
    the ring reservation honest), and each chunk is single-parity so the
    aggregation matmul reads the correct half of the gathered pair row.
  - Segment-sum per 128-dst block is a PE matmul with a one-hot selection
    matrix built on DVE via is_equal against an iota row (pad dstloc=-1 rows
    select nothing); PSUM accumulates across message chunks.
  - Epilogues apply dinv/bias/relu and chain directly into the next layer's
    paired table transform. Final output is produced transposed and unsharded
    on host.

The Bass program is identical on all cores (SPMD); chunk counts per
(block, parity) are the max over cores.
"""

import math
import sys

sys.path.insert(0, "/opt/trn_rl_repo")

import ml_dtypes
import numpy as np


# ---------------------------------------------------------------------------
# configuration
# ---------------------------------------------------------------------------
class Cfg:
    CORES = 8
    N = 100000
    IN_C = 128
    HID = 64
    OUT_C = 40
    NPC = 12500  # nodes per core
    NPC_PAD = 12544  # = 98 * 128
    BLK = 128
    SBB = 2  # dst blocks per superblock (dinv-load granularity)
    SG = 8  # chunks per S-build op
    MAXCH = 5  # max chunks per dma_gather call
    WIN = 32768  # int16 index reach (pair rows) per gather call

    @property
    def NBLK(self):
        return self.NPC_PAD // self.BLK

    @property
    def NSB(self):
        return math.ceil(self.NBLK / self.SBB)

    @property
    def TAB(self):
        return self.NPC_PAD * self.CORES

    @property
    def TAB2(self):
        return self.TAB // 2  # pair rows


# ---------------------------------------------------------------------------
# host-side prep: shard edges, build shared static schedule + per-core arrays
# ---------------------------------------------------------------------------
def _prepare(cfg, edge_index):
    src = np.asarray(edge_index[0], dtype=np.int64)
    dst = np.asarray(edge_index[1], dtype=np.int64)
    loop = np.arange(cfg.N, dtype=np.int64)

    # degree includes the appended self-loops, but the loops themselves are
    # NOT scheduled as gather slots: each core adds its own staged shard rows
    # directly on the PE (constant even/odd spread matmuls per block).
    deg = np.bincount(dst, minlength=cfg.N).astype(np.float32)
    deg += 1.0

    owner = dst // cfg.NPC
    dl_all = dst - owner * cfg.NPC
    srow_all = (src // cfg.NPC) * cfg.NPC_PAD + (src % cfg.NPC)
    par_all = srow_all & 1
    prow_all = srow_all >> 1
    blk_all = dl_all // cfg.BLK

    # per (core, block, parity): edges sorted by source pair-row so each
    # 128-slot chunk covers a narrow table window (per-call dynamic base keeps
    # int16 reach) and per-core pads cluster at bin tails (idx=-1 -> skipped).
    per_cbp = [
        [[None, None] for _ in range(cfg.NBLK)] for _ in range(cfg.CORES)
    ]
    counts = np.zeros((cfg.CORES, cfg.NBLK, 2), dtype=np.int64)
    for c in range(cfg.CORES):
        m = owner == c
        prow, dl, blk, par = prow_all[m], dl_all[m], blk_all[m], par_all[m]
        key = blk * 2 + par
        order = np.lexsort((prow, key))
        prow, dl, key = prow[order], dl[order], key[order]
        bounds = np.searchsorted(key, np.arange(2 * cfg.NBLK + 1))
        for b in range(cfg.NBLK):
            for p in range(2):
                lo, hi = bounds[2 * b + p], bounds[2 * b + p + 1]
                per_cbp[c][b][p] = (prow[lo:hi], dl[lo:hi] % cfg.BLK)
                counts[c, b, p] = hi - lo

    sched = np.ceil(counts.max(axis=0) / cfg.BLK).astype(np.int64)  # [NBLK,2]

    chunk_blocks = []
    chunk_par = []
    bins = []  # (b, p, first_chunk, n_chunks)
    for b in range(cfg.NBLK):
        for p in range(2):
            k = int(sched[b, p])
            bins.append((b, p, len(chunk_blocks), k))
            chunk_blocks.extend([b] * k)
            chunk_par.extend([p] * k)
    nchunk = len(chunk_blocks)
    nslot = nchunk * cfg.BLK

    rows_arr = np.full((cfg.CORES, nslot), -1, dtype=np.int64)
    dloc_arr = np.full((cfg.CORES, nslot), -1, dtype=np.int64)
    for b, p, j0, k in bins:
        pos = j0 * cfg.BLK
        for c in range(cfg.CORES):
            prow, dloc = per_cbp[c][b][p]
            n = len(prow)
            rows_arr[c, pos : pos + n] = prow
            dloc_arr[c, pos : pos + n] = dloc
    del per_cbp

    # greedy call formation within each (block, parity) bin
    calls = []  # (base, first_chunk, n_chunks)
    for b, p, j0, k in bins:
        j = j0
        jend = j0 + k
        while j < jend:
            lo_all, hi_all = None, None
            take = 0
            while take < cfg.MAXCH and j + take < jend:
                s = (j + take) * cfg.BLK
                seg = rows_arr[:, s : s + cfg.BLK]
                seg = seg[seg >= 0]
                if len(seg) == 0:
                    take += 1
                    continue
                lo = min(lo_all, seg.min()) if lo_all is not None else seg.min()
                hi = max(hi_all, seg.max()) if hi_all is not None else seg.max()
                if hi - lo >= cfg.WIN:
                    break
                lo_all, hi_all = lo, hi
                take += 1
            assert take > 0, "single chunk exceeds gather window"
            base = int(lo_all) if lo_all is not None else 0
            calls.append((base, j, take))
            s0, s1 = j * cfg.BLK, (j + take) * cfg.BLK
            seg = rows_arr[:, s0:s1]
            np.subtract(seg, base, out=seg, where=seg >= 0)
            j += take

    assert rows_arr.max() < cfg.WIN
    idx_maps = []
    dstloc_maps = []
    gcnt_maps = []
    for c in range(cfg.CORES):
        idx_arr = rows_arr[c]
        idx_maps.append(np.tile(idx_arr.astype(np.int16).reshape(-1, 16).T, (8, 1)))
        dstloc_maps.append(
            np.ascontiguousarray(
                dloc_arr[c].astype(np.float32).reshape(nchunk, cfg.BLK).T
            )
        )
        # per-call count of real (non-negative) slots: the ucode trims the
        # negative tail, and the decode-side ring reservation must match what
        # the Q7 actually writes -> num_idxs_reg must equal this count.
        cnt = np.empty(len(calls), dtype=np.uint32)
        for k, (_, j0, nch) in enumerate(calls):
            seg = idx_arr[j0 * cfg.BLK : (j0 + nch) * cfg.BLK]
            cnt[k] = int((seg >= 0).sum())
        gcnt_maps.append(cnt.reshape(1, -1))

    # greedy least-loaded SWDGE queue assignment (descgen-bound pairs: span
    # follows the most-loaded pair; plain ci%4 leaves ~8% imbalance). Never
    # assign two consecutive calls to the same pair so dispatch pipelines.
    mean_cnt = np.mean([g[0] for g in gcnt_maps], axis=0)
    loads = np.zeros(4)
    qns = []
    last = -1
    for c in mean_cnt:
        order = np.argsort(loads)
        pick = int(order[0]) if int(order[0]) != last else int(order[1])
        qns.append(pick)
        loads[pick] += c
        last = pick

    return {
        "qns": qns,
        "deg": deg,
        "sched": sched,
        "chunk_blocks": chunk_blocks,
        "chunk_par": chunk_par,
        "calls": calls,
        "nchunk": nchunk,
        "nslot": nslot,
        "idx_maps": idx_maps,
        "dstloc_maps": dstloc_maps,
        "gcnt_maps": gcnt_maps,
    }


# ---------------------------------------------------------------------------
# device program
# ---------------------------------------------------------------------------
def _build(cfg, chunk_blocks, chunk_par, calls, qns, debug=False, taps=False):
    import concourse.bacc as bacc
    import concourse.mybir as mybir
    import concourse.tile as tile
    from concourse import library_config

    fp32 = mybir.dt.float32
    bf16 = mybir.dt.bfloat16
    AF = mybir.ActivationFunctionType
    ALU = mybir.AluOpType

    nchunk = len(chunk_blocks)
    first_chunk = {}
    last_chunk = {}
    for j, b in enumerate(chunk_blocks):
        first_chunk.setdefault(b, j)
        last_chunk[b] = j
    max_call_ch = max(n for _, _, n in calls)
    HP = cfg.HID // 2  # 32: pair partitions per... (pairs per block = 64)
    PPB = cfg.BLK // 2  # 64 pairs per block

    nc = bacc.Bacc(
        "TRN2",
        target_bir_lowering=False,
        debug=debug,
        num_swdge_queues=4,
        dynamic_dma_scratch_size=32768,
    )

    xT_in = nc.dram_tensor("xTpe", [cfg.IN_C, cfg.NPC_PAD], bf16, kind="ExternalInput")
    W1_in = nc.dram_tensor("W1", [cfg.IN_C, cfg.HID], bf16, kind="ExternalInput")
    W2p_in = nc.dram_tensor("W2p", [cfg.HID, cfg.HID], fp32, kind="ExternalInput")
    b1_in = nc.dram_tensor("b1c", [cfg.HID, 1], fp32, kind="ExternalInput")
    b2_in = nc.dram_tensor("b2c", [cfg.OUT_C, 1], fp32, kind="ExternalInput")
    # dinv broadcast across 128 partitions, node order (for epilogues)
    dinvbc_in = nc.dram_tensor(
        "dinv_bc", [128, cfg.NPC_PAD], fp32, kind="ExternalInput"
    )
    # dinv in pair layout [64 pairs, even|odd], broadcast on 64 partitions,
    # per block 128 wide (for the layer-1 table transform)
    dinvpw_in = nc.dram_tensor(
        "dinv_pw", [PPB, cfg.NPC_PAD], fp32, kind="ExternalInput"
    )
    idx_in = nc.dram_tensor(
        "idxs", [128, (nchunk * cfg.BLK) // 16], mybir.dt.int16, kind="ExternalInput"
    )
    dstloc_in = nc.dram_tensor(
        "dstloc", [cfg.BLK, nchunk], bf16, kind="ExternalInput"
    )
    gcnt_in = nc.dram_tensor(
        "gcnt", [1, len(calls)], mybir.dt.uint32, kind="ExternalInput"
    )
    out_t = nc.dram_tensor(
        "outT", [cfg.OUT_C, cfg.NPC_PAD], fp32, kind="ExternalOutput"
    )
    if taps:
        tap1 = nc.dram_tensor(
            "tap1", [cfg.NPC_PAD // 2, cfg.BLK], bf16, kind="ExternalOutput"
        )
        tap2 = nc.dram_tensor(
            "tap2", [cfg.NPC_PAD // 2, cfg.BLK], bf16, kind="ExternalOutput"
        )

    shard1 = nc.dram_tensor("shard1", [cfg.NPC_PAD // 2, cfg.BLK], bf16)
    shard2 = nc.dram_tensor("shard2", [cfg.NPC_PAD // 2, cfg.BLK], bf16)
    table1 = nc.dram_tensor("table1", [cfg.TAB2, cfg.BLK], bf16, addr_space="Shared")
    table2 = nc.dram_tensor("table2", [cfg.TAB2, cfg.BLK], bf16, addr_space="Shared")
    iota_c = nc.inline_tensor(
        np.tile(np.arange(cfg.BLK, dtype=np.float32), (128, cfg.SG))
        .reshape(128, cfg.SG * cfg.BLK)
        .astype(ml_dtypes.bfloat16),
        name="iota_sg",
    )
    # constant spread matrices for the self-loop contribution: pair row p of
    # the staged shard feeds dst column 2p (even half) / 2p+1 (odd half)
    se_np = np.zeros((PPB, cfg.BLK), np.float32)
    se_np[np.arange(PPB), 2 * np.arange(PPB)] = 1.0
    so_np = np.zeros((PPB, cfg.BLK), np.float32)
    so_np[np.arange(PPB), 2 * np.arange(PPB) + 1] = 1.0
    se_c = nc.inline_tensor(se_np.astype(ml_dtypes.bfloat16), name="spread_even")
    so_c = nc.inline_tensor(so_np.astype(ml_dtypes.bfloat16), name="spread_odd")

    replica = [list(range(cfg.CORES))]

    with tile.TileContext(nc) as tc:
        with (
            tc.tile_pool(name="cst", bufs=1) as cst,
            tc.tile_pool(name="gbp", bufs=10) as gbp,
            tc.tile_pool(name="sp", bufs=4) as sp,
            tc.tile_pool(name="dv", bufs=2) as dv,
            tc.tile_pool(name="ev", bufs=6) as ev,
        ):
            nc.gpsimd.load_library(library_config.mlp)

            # pre-zero the gather pool: tail slots skipped by descgen leave
            # stale tile data that meets a zero S column; NaN/Inf from
            # uninitialized SBUF would still poison 0*NaN, zeros can't.
            for _ in range(10):
                gbt = gbp.tile([128, max_call_ch, cfg.BLK], bf16, tag="gb")
                nc.vector.memset(gbt[:], 0.0)

            # ---- constants ----
            W1t = cst.tile([cfg.IN_C, cfg.HID], bf16)
            nc.sync.dma_start(W1t[:], W1_in[:])
            W2t = cst.tile([cfg.HID, cfg.HID], fp32)
            nc.sync.dma_start(W2t[:], W2p_in[:])
            b1t = cst.tile([cfg.HID, 1], fp32)
            nc.sync.dma_start(b1t[:], b1_in[:])
            b2t = cst.tile([cfg.OUT_C, 1], fp32)
            nc.sync.dma_start(b2t[:], b2_in[:])
            iota = cst.tile([128, cfg.SG * cfg.BLK], bf16)
            nc.sync.dma_start(iota[:], iota_c[:])
            idxt = cst.tile([128, (nchunk * cfg.BLK) // 16], mybir.dt.int16)
            nc.sync.dma_start(idxt[:], idx_in[:])
            dstloct = cst.tile([cfg.BLK, nchunk], bf16)
            nc.sync.dma_start(dstloct[:], dstloc_in[:])
            gcntt = cst.tile([1, len(calls)], mybir.dt.uint32)
            nc.sync.dma_start(gcntt[:], gcnt_in[:])
            set_ = cst.tile([PPB, cfg.BLK], bf16)
            nc.sync.dma_start(set_[:], se_c[:])
            sot = cst.tile([PPB, cfg.BLK], bf16)
            nc.sync.dma_start(sot[:], so_c[:])
            cnt_regs = [nc.gpsimd.alloc_register(f"gather_cnt{i}") for i in range(16)]

            # ---- aggregation layer (shared for both layers) ----
            def agg_layer(layer, table, stag2, stag_self, shard_out=None):
                ch_out = cfg.HID if layer == 1 else cfg.OUT_C
                s_tiles = {}

                def s_for(j):
                    gi = j // cfg.SG
                    if gi not in s_tiles:
                        n = min(cfg.SG, nchunk - gi * cfg.SG)
                        st = sp.tile([128, cfg.SG * cfg.BLK], bf16, tag="s")
                        nc.vector.tensor_tensor(
                            out=st[:].rearrange("p (a b) -> p a b", b=cfg.BLK)[
                                :, :n, :
                            ],
                            in0=iota[:].rearrange("p (a b) -> p a b", b=cfg.BLK)[
                                :, :n, :
                            ],
                            in1=dstloct[:, gi * cfg.SG : gi * cfg.SG + n].to_broadcast(
                                [128, n, cfg.BLK]
                            ),
                            op=ALU.is_equal,
                        )
                        s_tiles[gi] = st
                    return s_tiles[gi], (j % cfg.SG)

                psums = {}
                ci = 0  # call cursor
                for sb in range(cfg.NSB):
                    blo, bhi = sb * cfg.SBB, min((sb + 1) * cfg.SBB, cfg.NBLK)
                    nsb = (bhi - blo) * cfg.BLK
                    dinvrep = dv.tile([128, cfg.SBB * cfg.BLK], fp32, tag="dr")
                    nc.sync.dma_start(
                        dinvrep[:, :nsb],
                        dinvbc_in[:, blo * cfg.BLK : blo * cfg.BLK + nsb],
                    )

                    while ci < len(calls):
                        base, j0, nch = calls[ci]
                        if chunk_blocks[j0] >= bhi:
                            break
                        qn = qns[ci]
                        k = ci
                        ci += 1
                        if k % 16 == 0:
                            n = min(16, len(calls) - k)
                            nc.gpsimd.reg_load(cnt_regs[:n], gcntt[0:1, k : k + n])
                        rows = min(cfg.WIN, cfg.TAB2 - base)
                        gbt = gbp.tile([128, max_call_ch, cfg.BLK], bf16, tag="gb")
                        nc.gpsimd.dma_gather(
                            gbt[:, :nch, :],
                            table[base : base + rows, :],
                            idxt[:, (j0 * cfg.BLK) // 16 : ((j0 + nch) * cfg.BLK) // 16],
                            nch * cfg.BLK,
                            cnt_regs[k % 16],
                            cfg.BLK,
                            queue_num=qn,
                        )
                        for j in range(j0, j0 + nch):
                            b = chunk_blocks[j]
                            if b not in psums:
                                pstile = tc_psum.tile(
                                    [ch_out, cfg.BLK], fp32, tag=f"ps{layer}"
                                )
                                psums[b] = pstile
                                # self-loop contribution from the local shard
                                nc.tensor.matmul(
                                    pstile[:],
                                    lhsT=stag_self[:, b, 0:ch_out],
                                    rhs=set_[:],
                                    start=True,
                                    stop=False,
                                )
                                nc.tensor.matmul(
                                    pstile[:],
                                    lhsT=stag_self[:, b, cfg.HID : cfg.HID + ch_out],
                                    rhs=sot[:],
                                    start=False,
                                    stop=False,
                                )
                            st, kk = s_for(j)
                            off = cfg.HID * chunk_par[j]
                            nc.tensor.matmul(
                                psums[b][:],
                                lhsT=gbt[:, j - j0, off : off + ch_out],
                                rhs=st[:, kk * cfg.BLK : (kk + 1) * cfg.BLK],
                                start=False,
                                stop=(j == last_chunk[b]),
                            )

                    # epilogues for this superblock's blocks
                    for b in range(blo, bhi):
                        off = (b - blo) * cfg.BLK
                        ps = psums.pop(b)
                        if layer == 1:
                            t1 = ev.tile([cfg.HID, cfg.BLK], fp32, tag="t1")
                            nc.vector.tensor_tensor(
                                out=t1[:],
                                in0=ps[:],
                                in1=dinvrep[: cfg.HID, off : off + cfg.BLK],
                                op=ALU.mult,
                            )
                            hr = ev.tile([cfg.HID, cfg.BLK], fp32, tag="hr")
                            nc.scalar.activation(hr[:], t1[:], AF.Relu, bias=b1t[:])
                            # gb columns reordered [even dsts | odd dsts] so the
                            # paired table transform is two contiguous matmuls
                            gb = ev.tile([cfg.HID, cfg.BLK], fp32, tag="gblk")
                            nc.vector.tensor_tensor(
                                out=gb[:, :PPB],
                                in0=hr[:, 0 : cfg.BLK : 2],
                                in1=dinvrep[: cfg.HID, off : off + cfg.BLK : 2],
                                op=ALU.mult,
                            )
                            nc.vector.tensor_tensor(
                                out=gb[:, PPB:],
                                in0=hr[:, 1 : cfg.BLK : 2],
                                in1=dinvrep[: cfg.HID, off + 1 : off + cfg.BLK : 2],
                                op=ALU.mult,
                            )
                            ps2 = tc_ps2.tile([PPB, cfg.BLK], fp32, tag="ps2")
                            nc.tensor.matmul(
                                ps2[:, : cfg.HID],
                                lhsT=gb[:, :PPB],
                                rhs=W2t[:],
                                start=True,
                                stop=True,
                            )
                            nc.tensor.matmul(
                                ps2[:, cfg.HID :],
                                lhsT=gb[:, PPB:],
                                rhs=W2t[:],
                                start=True,
                                stop=True,
                            )
                            nc.scalar.activation(
                                stag2[:, b, :], ps2[:], AF.Copy, bias=0.0
                            )
                        else:
                            t1 = ev.tile([cfg.OUT_C, cfg.BLK], fp32, tag="t2")
                            nc.vector.tensor_tensor(
                                out=t1[:],
                                in0=ps[:],
                                in1=dinvrep[: cfg.OUT_C, off : off + cfg.BLK],
                                op=ALU.mult,
                            )
                            nc.scalar.activation(
                                stag2[:, b * cfg.BLK : (b + 1) * cfg.BLK],
                                t1[:],
                                AF.Identity,
                                bias=b2t[:],
                            )
                    if shard_out is not None:
                        nc.sync.dma_start(
                            shard_out[:, blo:bhi, :], stag2[:, blo:bhi, :]
                        )

            with tc.tile_pool(name="stg2", bufs=1) as st2p:
                stag2 = st2p.tile([PPB, cfg.NBLK, cfg.BLK], bf16)

                with tc.tile_pool(name="stg1", bufs=1) as st1:
                    # ---- layer-1 transform: shard1 pairs = dinv * (x@W1) ----
                    stag1 = st1.tile([PPB, cfg.NBLK, cfg.BLK], bf16)
                    with (
                        tc.tile_pool(name="phA", bufs=3) as pa,
                        tc.tile_pool(name="dpw", bufs=1) as dpw,
                        tc.tile_pool(name="psA", bufs=4, space="PSUM") as psA,
                    ):
                        dpwt = dpw.tile([PPB, cfg.NPC_PAD], fp32)
                        nc.sync.dma_start(dpwt[:], dinvpw_in[:])
                        sh1v = shard1.rearrange("(b p) d -> p b d", p=PPB)
                        for b in range(cfg.NBLK):
                            if b % 8 == 0:
                                nbb = min(8, cfg.NBLK - b)
                                xc8 = pa.tile([cfg.IN_C, 8 * cfg.BLK], bf16)
                                nc.sync.dma_start(
                                    xc8[:, : nbb * cfg.BLK],
                                    xT_in[
                                        :,
                                        b * cfg.BLK : (b + nbb) * cfg.BLK,
                                    ],
                                )
                            xo = (b % 8) * cfg.BLK
                            ps = psA.tile([PPB, cfg.BLK], fp32)
                            nc.tensor.matmul(
                                ps[:, : cfg.HID],
                                lhsT=xc8[:, xo : xo + PPB],
                                rhs=W1t[:],
                                start=True,
                                stop=True,
                            )
                            nc.tensor.matmul(
                                ps[:, cfg.HID :],
                                lhsT=xc8[:, xo + PPB : xo + cfg.BLK],
                                rhs=W1t[:],
                                start=True,
                                stop=True,
                            )
                            nc.vector.tensor_tensor(
                                out=stag1[:, b, :],
                                in0=ps[:],
                                in1=dpwt[:, b * cfg.BLK : (b + 1) * cfg.BLK],
                                op=ALU.mult,
                            )
                            # stream the shard out per superblock so the
                            # collective can start right after the last block
                            if b % cfg.SBB == cfg.SBB - 1 or b == cfg.NBLK - 1:
                                blo = (b // cfg.SBB) * cfg.SBB
                                nc.sync.dma_start(
                                    sh1v[:, blo : b + 1, :],
                                    stag1[:, blo : b + 1, :],
                                )

                    nc.gpsimd.collective_compute(
                        "AllGather",
                        mybir.AluOpType.bypass,
                        replica_groups=replica,
                        ins=[shard1[:]],
                        outs=[table1[:]],
                    )
                    if taps:
                        nc.sync.dma_start(tap1[:], shard1[:])

                    # layer 1 aggregation (+ paired table2 transform fused)
                    with (
                        tc.tile_pool(name="ps2p", bufs=2, space="PSUM") as tc_ps2,
                        tc.tile_pool(name="psagg1", bufs=6, space="PSUM") as tc_psum,
                    ):
                        agg_layer(
                            1,
                            table1,
                            stag2,
                            stag1,
                            shard_out=shard2.rearrange("(b p) d -> p b d", p=PPB),
                        )

                nc.gpsimd.collective_compute(
                    "AllGather",
                    mybir.AluOpType.bypass,
                    replica_groups=replica,
                    ins=[shard2[:]],
                    outs=[table2[:]],
                )
                if taps:
                    nc.sync.dma_start(tap2[:], shard2[:])

                # layer 2 aggregation -> transposed output
                with (
                    tc.tile_pool(name="outp", bufs=1) as outp,
                    tc.tile_pool(name="psagg2", bufs=6, space="PSUM") as tc_psum,
                    tc.tile_pool(name="ps2p2", bufs=1, space="PSUM") as tc_ps2,
                ):
                    outT = outp.tile([cfg.OUT_C, cfg.NPC_PAD], fp32)
                    agg_layer(2, table2, outT, stag2)
                    nc.sync.dma_start(out_t[:], outT[:])

    nc.compile()
    return nc


# ---------------------------------------------------------------------------
# public entry point
# ---------------------------------------------------------------------------
def _make_in_maps(cfg, prep, x, W1, b1, W2, b2):
    PPB = cfg.BLK // 2
    W2p = np.zeros((cfg.HID, cfg.HID), np.float32)
    W2p[:, : cfg.OUT_C] = W2
    deg = prep["deg"]
    in_maps = []
    for c in range(cfg.CORES):
        xs = x[c * cfg.NPC : (c + 1) * cfg.NPC]  # [NPC, IN_C]
        xT = np.zeros((cfg.IN_C, cfg.NPC_PAD), np.float32)
        xT[:, : cfg.NPC] = xs.T
        # per-block column permutation [even nodes | odd nodes] so the paired
        # layer-1 transform uses contiguous lhsT slices
        xT_pe = (
            xT.reshape(cfg.IN_C, cfg.NBLK, PPB, 2)
            .transpose(0, 1, 3, 2)
            .reshape(cfg.IN_C, cfg.NPC_PAD)
        )
        # pad nodes: huge degree -> dinv ~ 0 -> pad table rows ~ 0
        dg = np.full(cfg.NPC_PAD, 1e30, np.float32)
        dg[: cfg.NPC] = deg[c * cfg.NPC : (c + 1) * cfg.NPC]
        dinv = (1.0 / np.sqrt(dg)).astype(np.float32)
        dinv_bc = np.ascontiguousarray(
            np.broadcast_to(dinv[None, :], (128, cfg.NPC_PAD))
        )
        # scale tile for the paired layer-1 transform: partition kk = pair-in-
        # block, columns [even-half | odd-half]: value = dinv(block*128 + 2kk
        # + (col >= 64))
        dpair = dinv.reshape(cfg.NBLK, PPB, 2)  # [b, pair, half]
        dinv_pw = np.empty((PPB, cfg.NBLK, 2, PPB), np.float32)
        for h in range(2):
            # [pair kk, block, half h, any col] = dpair[b, kk, h]
            dinv_pw[:, :, h, :] = dpair[:, :, h].T[:, :, None]
        dinv_pw = np.ascontiguousarray(dinv_pw.reshape(PPB, cfg.NPC_PAD))
        in_maps.append(
            {
                "xTpe": np.ascontiguousarray(xT_pe).astype(ml_dtypes.bfloat16),
                "W1": np.asarray(W1, np.float32).astype(ml_dtypes.bfloat16),
                "W2p": W2p,
                "b1c": np.asarray(b1, np.float32).reshape(cfg.HID, 1),
                "b2c": np.asarray(b2, np.float32).reshape(cfg.OUT_C, 1),
                "dinv_bc": dinv_bc,
                "dinv_pw": dinv_pw,
                "idxs": prep["idx_maps"][c],
                "dstloc": prep["dstloc_maps"][c].astype(ml_dtypes.bfloat16),
                "gcnt": prep["gcnt_maps"][c],
            }
        )
    return in_maps


def _run(cfg, inputs, mode="hw", trace=False, taps=False):
    x = np.asarray(inputs["x"], np.float32)
    edge_index = np.asarray(inputs["edge_index"])
    W1 = np.asarray(inputs["W1"], np.float32)
    b1 = np.asarray(inputs["b1"], np.float32)
    W2 = np.asarray(inputs["W2"], np.float32)
    b2 = np.asarray(inputs["b2"], np.float32)

    prep = _prepare(cfg, edge_index)
    nc = _build(
        cfg,
        prep["chunk_blocks"],
        prep["chunk_par"],
        prep["calls"],
        prep["qns"],
        debug=(mode == "sim"),
        taps=taps,
    )
    in_maps = _make_in_maps(cfg, prep, x, W1, b1, W2, b2)

    info = {}
    if mode == "sim":
        from concourse.bass_interp import MultiCoreSim

        sim = MultiCoreSim(nc, cfg.CORES)
        for c in range(cfg.CORES):
            for k, v in in_maps[c].items():
                sim.cores[c].tensor(k)[:] = v
        sim.simulate()
        outs = [sim.cores[c].tensor("outT").copy() for c in range(cfg.CORES)]
        if taps:
            info["taps"] = [
                {k: sim.cores[c].tensor(k).copy() for k in ("tap1", "tap2")}
                for c in range(cfg.CORES)
            ]
    else:
        import concourse.bass_utils as bu

        if trace:
            # avoid the S3 artifact upload in the profile path
            bu.upload_artifacts = lambda d: "(local)"
        r = bu.run_bass_kernel_spmd(
            nc,
            in_maps,
            list(range(cfg.CORES)),
            trace=trace,
            tmpdir=(inputs.get("_tracedir") if trace else None),
        )
        info["exec_time_ns"] = r.exec_time_ns
        info["mean_exec_time_ns"] = r.mean_exec_time_ns
        outs = [r.results[c]["outT"] for c in range(cfg.CORES)]
        if taps:
            info["taps"] = [
                {k: r.results[c][k] for k in ("tap1", "tap2")}
                for c in range(cfg.CORES)
            ]

    out = np.concatenate([o[:, : cfg.NPC].T for o in outs], axis=0)
    return out.astype(np.float32), info


def kernel(**inputs):
    out, _ = _run(Cfg(), inputs, mode="hw")
    return out
